# revision 62
# baseline (speedup 1.0000x reference)
"""Trainium2 Bass kernel for EnhancedMultiHeadAttention (B=2, S=2048, DM=1024, H=16).

Sharding: 8 NeuronCores = 2 batches x 4 query-row blocks of 512 rows. Each
core computes K/V for its whole batch (4x redundant: cheaper than sharing via
collectives, which measured ~45GB/s with heavy serialization on the CC path),
plus attention, output projection, gate and layernorm for its own 512 query
rows. No collectives; the host concatenates the 8 output shards.

All matmuls run in bf16 with fp32 PSUM accumulation (rel err ~8e-4, gate is
2e-2). bf16 halves input DMA bytes, lets fast-weight-load halve LDWEIGHTS,
and lets the row/col tile_position-packed score and attn@v matmul pairs
actually overlap on the PE (fp32r pairs serialized; bf16 pairs run with the
second MM finishing ~30ns after the first).

Schedule: one fused window. Q^T is projected up front; each head pair's K
rows are projected just-in-time; V tiles are projected into SBUF-resident
tiles, with pair 0's V tiles fused into its attention t-loop so the ScalarE
exact-erf GELU stream (~130us serial: 16.8M elems/core) starts as early as
possible. All projection matmuls fill PE slack underneath the GELU stream.
The score scale rides the GELU activation's free affine;
softmax(attention_weights) is folded into Wv/bv on the host; per-core query
columns are permuted to the front of x^T so Q projects from the resident
tile (t-order in attention is permutation-invariant as long as K and V share
it). The gate projection reads ctx^T directly via the host-folded
Wgo = Wg @ Wo (bgo = bg + Wg bo), which removes the transposed
out-projection of the previous version entirely. Epilogue (out proj via
ctx^T, gate, sigmoid blend, bn_stats layernorm) trails per 128-row tile.
"""
import math
import sys

import numpy as np

for _p in ("/opt/trn_rl_repo", "/opt/pypackages"):
    if _p not in sys.path:
        sys.path.append(_p)

import ml_dtypes
import concourse.bass as bass
import concourse.mybir as mybir
import concourse.tile as tile
from concourse import bacc
from concourse.bass_utils import run_bass_kernel_spmd

F32 = mybir.dt.float32
BF16 = mybir.dt.bfloat16
FP8 = mybir.dt.float8e4
DR = mybir.MatmulPerfMode.DoubleRow
AF = mybir.ActivationFunctionType
ALU = mybir.AluOpType

B, S, DM, H = 2, 2048, 1024, 16
HD = DM // H                  # 64
SQ = 512                      # query rows per core
NP = 128                      # partitions
KC = DM // NP                 # 8 contraction chunks
NT = S // NP                  # 16 key/value tiles
NPAIR = H // 2                # 8 head pairs
NST = SQ // NP                # 4 row tiles in epilogue
N512 = 512
NQ = 512                      # v-projection column half width
SCALE = 1.0 / math.sqrt(HD)
EPS = 1e-5

_CACHE = {}
_TRACE = [False]
_LAST_RESULT = [None]


def _bcast(ap_1d, p=NP):
    return bass.AP(tensor=ap_1d.tensor, offset=ap_1d.offset,
                   ap=[[0, p]] + list(ap_1d.ap))


def _build():
    nc = bacc.Bacc("TRN2", target_bir_lowering=False, debug=False)

    # all big inputs are host-laid-out p-major ("(p c) ..." rows) so each
    # DMA needs only one contiguous descriptor per partition
    xT8q_d = nc.dram_tensor("xT8q", [DM, SQ], FP8, kind="ExternalInput").ap()
    xT8r_d = nc.dram_tensor("xT8r", [DM, S - SQ], FP8,
                            kind="ExternalInput").ap()
    xr_d = nc.dram_tensor("xr", [SQ, DM], BF16, kind="ExternalInput").ap()
    wk8_d = nc.dram_tensor("wk8", [DM, DM], FP8, kind="ExternalInput").ap()
    wv8_d = nc.dram_tensor("wv8", [DM, DM], FP8, kind="ExternalInput").ap()
    wq8_d = nc.dram_tensor("wq8", [DM, DM], FP8, kind="ExternalInput").ap()
    gsc_d = nc.dram_tensor("gsc", [1], F32, kind="ExternalInput").ap()
    oscv_d = nc.dram_tensor("oscv", [3], F32, kind="ExternalInput").ap()
    wo8_d = nc.dram_tensor("wo8", [DM, DM], FP8, kind="ExternalInput").ap()
    wgo8_d = nc.dram_tensor("wgo8", [DM, DM], FP8, kind="ExternalInput").ap()
    bq_d = nc.dram_tensor("bq", [DM], F32, kind="ExternalInput").ap()
    bk_d = nc.dram_tensor("bk", [DM], F32, kind="ExternalInput").ap()
    bv_d = nc.dram_tensor("bv", [DM], F32, kind="ExternalInput").ap()
    bo_d = nc.dram_tensor("bo", [DM], F32, kind="ExternalInput").ap()
    bgo_d = nc.dram_tensor("bgo", [DM], F32, kind="ExternalInput").ap()
    gam_d = nc.dram_tensor("gam", [DM], F32, kind="ExternalInput").ap()
    bet_d = nc.dram_tensor("bet", [DM], F32, kind="ExternalInput").ap()
    y_d = nc.dram_tensor("y", [SQ, DM], F32, kind="ExternalOutput").ap()

    # DoubleRow row order (p, kc2, two): contraction row = kc2*256+two*128+p
    xT8q_v = xT8q_d.rearrange("(p k two) s -> p k two s",
                              p=NP, k=KC // 2, two=2)
    xT8r_v = xT8r_d.rearrange("(p k two) s -> p k two s",
                              p=NP, k=KC // 2, two=2)
    wk8_v = wk8_d.rearrange("(p k two) d -> p k two d", p=NP, k=KC // 2, two=2)
    wv8_v = wv8_d.rearrange("(p k two) d -> p k two d", p=NP, k=KC // 2, two=2)
    wq8_v = wq8_d.rearrange("(p k two) d -> p k two d", p=NP, k=KC // 2, two=2)
    wo8_v = wo8_d.rearrange("(p k two) d -> p k two d", p=NP, k=KC // 2, two=2)
    wgo8_v = wgo8_d.rearrange("(p k two) d -> p k two d", p=NP, k=KC // 2, two=2)

    with tile.TileContext(nc) as tc:
        with tc.tile_pool(name="pers", bufs=1) as pers, \
             tc.tile_pool(name="acc", bufs=1) as acc:
            bq_sb = pers.tile([NP, KC], F32)
            bk_sb = pers.tile([NP, KC], F32)
            nc.sync.dma_start(out=bq_sb, in_=bq_d.rearrange("(p c) -> p c", p=NP))
            nc.sync.dma_start(out=bk_sb, in_=bk_d.rearrange("(p c) -> p c", p=NP))
            bv_bc = pers.tile([NP, DM], F32)
            nc.sync.dma_start(out=bv_bc, in_=_bcast(bv_d))
            gsc_sb = pers.tile([NP, 1], F32)
            nc.sync.dma_start(out=gsc_sb, in_=_bcast(gsc_d))
            # [csc, osc, sgsc]: ctx fp8 quantization scale, out-proj unscale,
            # gate-proj unscale (folded into the sigmoid's affine)
            osv_sb = pers.tile([NP, 3], F32)
            nc.sync.dma_start(out=osv_sb, in_=_bcast(oscv_d))
            eps_sb = pers.tile([NP, 1], F32)
            nc.vector.memset(eps_sb, EPS)

            ctxT_sb = acc.tile([NP, NPAIR, SQ], FP8)

            wep = tc.alloc_tile_pool(name="wep", bufs=1)
            wo8_sb = wep.tile([NP, KC // 2, 2, DM], FP8)
            wgo8_sb = wep.tile([NP, KC // 2, 2, DM], FP8)
            xr_sb = wep.tile([NP, NST, DM], BF16)
            bo_bc = wep.tile([NP, DM], F32)
            bgo_bc = wep.tile([NP, DM], F32)
            gam_bc = wep.tile([NP, DM], F32)
            bet_bc = wep.tile([NP, DM], F32)

            with tc.tile_pool(name="xres", bufs=1) as xres, \
                 tc.tile_pool(name="qres", bufs=1) as qres, \
                 tc.tile_pool(name="wts", bufs=1) as wts, \
                 tc.tile_pool(name="kpp", bufs=2) as kpp, \
                 tc.tile_pool(name="vqp", bufs=1) as vqp, \
                 tc.tile_pool(name="attp", bufs=3) as attp, \
                 tc.tile_pool(name="pp", bufs=1, space="PSUM") as pp, \
                 tc.tile_pool(name="scop", bufs=3, space="PSUM") as scop, \
                 tc.tile_pool(name="cxp", bufs=1, space="PSUM") as cxp:
                x8q_sb = xres.tile([NP, KC // 2, 2, SQ], FP8)
                x8r_sb = xres.tile([NP, KC // 2, 2, S - SQ], FP8)
                qT_sb = qres.tile([NP, KC, SQ], BF16)
                wq8_sb = wts.tile([NP, KC // 2, 2, DM], FP8)
                wk8_sb = wts.tile([NP, KC // 2, 2, DM], FP8)
                wv8_sb = wts.tile([NP, KC // 2, 2, DM], FP8)
                vq = vqp.tile([NP, NT, DM], BF16)

                def x8_t(k2, tt):
                    if tt < NST:
                        return x8q_sb[:, k2, :, tt * NP:(tt + 1) * NP]
                    o = (tt - NST) * NP
                    return x8r_sb[:, k2, :, o:o + NP]

                def x8_s(k2, ts):
                    if ts == 0:
                        return x8q_sb[:, k2, :, :]
                    o = (ts - 1) * N512
                    return x8r_sb[:, k2, :, o:o + N512]

                def v_tt(tt, ns):
                    # v'[t, d] = sum_k x[t, k] (s_v Wv[d, k]), fp8 DoubleRow
                    ps_t = pp.tile([NP, NQ], F32, tag="pj", name="ps_t")
                    for k2 in range(KC // 2):
                        nc.tensor.matmul(
                            ps_t,
                            x8_t(k2, tt),
                            wv8_sb[:, k2, :, ns * NQ:(ns + 1) * NQ],
                            start=(k2 == 0), stop=(k2 == KC // 2 - 1),
                            perf_mode=DR)
                    nc.vector.tensor_add(
                        vq[:, tt, ns * NQ:(ns + 1) * NQ], ps_t,
                        bv_bc[:, ns * NQ:(ns + 1) * NQ])

                kpairs = [None] * NPAIR

                def prep_ts(p, ts):
                    # one 512-col slab of kpair[d, t] = s_k(Wk x^T + bk)
                    if ts == 0:
                        kpairs[p] = kpp.tile([NP, S], BF16, tag="kp",
                                             name="kpair")
                    kpair = kpairs[p]
                    ps_t = pp.tile([NP, N512], F32, tag="pj", name="ps_t")
                    for k2 in range(KC // 2):
                        nc.tensor.matmul(
                            ps_t,
                            wk8_sb[:, k2, :, p * NP:(p + 1) * NP],
                            x8_s(k2, ts),
                            start=(k2 == 0), stop=(k2 == KC // 2 - 1),
                            perf_mode=DR)
                    nc.vector.tensor_scalar_add(
                        kpair[:, ts * N512:(ts + 1) * N512], ps_t,
                        bk_sb[:, p:p + 1])

                def qproj(dt):
                    ps_q = pp.tile([NP, SQ], F32, tag="pj", name="ps_q")
                    for k2 in range(KC // 2):
                        nc.tensor.matmul(ps_q,
                                         wq8_sb[:, k2, :, dt * NP:(dt + 1) * NP],
                                         x8q_sb[:, k2, :, :],
                                         start=(k2 == 0),
                                         stop=(k2 == KC // 2 - 1),
                                         perf_mode=DR)
                    nc.vector.tensor_scalar_add(qT_sb[:, dt, :], ps_q,
                                                bq_sb[:, dt:dt + 1])

                def attn(p, pre_t=None, fill=()):
                    # fill: projection-chain closures for FUTURE pairs,
                    # dispensed evenly across the t-loop so the in-order PE
                    # queue never starves the GELU stream at pair boundaries
                    fill = list(fill)
                    n0 = len(fill)
                    kpair = kpairs[p]
                    c0 = p * NP
                    ctx_ps = cxp.tile([NP, SQ], F32, tag="cx", name="ctx_ps")
                    for t in range(NT):
                        while len(fill) > n0 * (NT - 1 - t) // NT:
                            fill.pop(0)()
                        if pre_t is not None:
                            pre_t(t)
                        sco = scop.tile([NP, 2 * SQ], F32, tag="sc", name="sco")
                        nc.tensor.matmul(sco[:, 0:SQ],
                                         kpair[0:64, t * NP:(t + 1) * NP],
                                         qT_sb[0:64, p, :],
                                         start=True, stop=True,
                                         tile_position=(0, 0))
                        nc.tensor.matmul(sco[:, SQ:2 * SQ],
                                         kpair[64:NP, t * NP:(t + 1) * NP],
                                         qT_sb[64:NP, p, :],
                                         start=True, stop=True,
                                         tile_position=(64, 0))
                        att_t = attp.tile([NP, 2 * SQ], BF16, tag="at", name="att_t")
                        nc.scalar.activation(out=att_t, in_=sco, func=AF.Gelu,
                                             scale=gsc_sb[:, 0:1])
                        nc.tensor.matmul(ctx_ps[0:64, :], vq[:, t, c0:c0 + 64],
                                         att_t[:, 0:SQ],
                                         start=(t == 0), stop=(t == NT - 1),
                                         tile_position=(0, 0))
                        nc.tensor.matmul(ctx_ps[64:NP, :],
                                         vq[:, t, c0 + 64:c0 + NP],
                                         att_t[:, SQ:2 * SQ],
                                         start=(t == 0), stop=(t == NT - 1),
                                         tile_position=(0, 64))
                    # rescale s_v*ctx -> s_c*ctx so it fits fp8 for the
                    # DoubleRow out/gate projections
                    nc.vector.tensor_scalar_mul(ctxT_sb[:, p, :], ctx_ps,
                                                osv_sb[:, 0:1])

                # opening DMA order by need-time: Q-proj deps (query cols +
                # wq) first, then wk (pair 0's K), xT rest, wv. The epilogue
                # prefetch (wo/wgo/xr, needed only ~250us in) is issued from
                # the scalar queue mid-attention so its descriptors don't
                # contend with the critical loads.
                # single queue (sync/HWDGE) in exact need order, so the
                # per-queue descriptor FIFOs preserve the priority
                nc.sync.dma_start(out=wq8_sb, in_=wq8_v)
                nc.sync.dma_start(out=x8q_sb, in_=xT8q_v)
                nc.sync.dma_start(out=wk8_sb, in_=wk8_v)
                nc.sync.dma_start(out=x8r_sb, in_=xT8r_v)
                nc.sync.dma_start(out=wv8_sb, in_=wv8_v)

                # only pair 0's projections run up front; everything else is
                # dispensed as fillers inside earlier pairs' t-loops
                def mk(f, *a):
                    return lambda: f(*a)

                qproj(0)
                for _ts in range(4):
                    prep_ts(0, _ts)
                attn(0, pre_t=lambda t: v_tt(t, 0),
                     fill=[mk(qproj, 1)] + [mk(prep_ts, 1, ts)
                                            for ts in range(4)])
                for p in range(1, NPAIR):
                    fl = []
                    if 1 <= p <= 4:
                        fl += [mk(v_tt, tt, 1)
                               for tt in range(4 * (p - 1), 4 * p)]
                    if p + 1 < NPAIR:
                        fl += [mk(qproj, p + 1)] + [mk(prep_ts, p + 1, ts)
                                                    for ts in range(4)]
                    if p == 4:
                        # epilogue prefetch: issued on the scalar queue, deep
                        # in the GELU stream by now — a natural throttle that
                        # keeps these loads off the DMA engines early on
                        nc.scalar.dma_start(out=wo8_sb, in_=wo8_v)
                        nc.scalar.dma_start(out=wgo8_sb, in_=wgo8_v)
                        nc.scalar.dma_start(
                            out=xr_sb,
                            in_=xr_d.rearrange("(n p) d -> p n d", p=NP))
                        nc.scalar.dma_start(out=bo_bc, in_=_bcast(bo_d))
                        nc.scalar.dma_start(out=bgo_bc, in_=_bcast(bgo_d))
                        nc.scalar.dma_start(out=gam_bc, in_=_bcast(gam_d))
                        nc.scalar.dma_start(out=bet_bc, in_=_bcast(bet_d))
                    attn(p, fill=fl)

            # ------------- out proj, gate + epilogue -----------------------
            with tc.tile_pool(name="big", bufs=1) as big, \
                 tc.tile_pool(name="pop", bufs=4, space="PSUM") as pop:
                orow_sb = big.tile([NP, NST, DM], BF16)
                gate_sb = big.tile([NP, NST, DM], BF16)
                t1_sb = big.tile([NP, NST, DM], BF16)
                y_sb = big.tile([NP, 2, DM], F32)
                stats = pers.tile([NP, 2, 6], F32)
                mv = pers.tile([NP, 2], F32)
                std = pers.tile([NP, 1], F32)
                rstd = pers.tile([NP, 1], F32)
                # all gate matmuls + sigmoids first, then the out-proj/LN
                # pass: SIGMOID and SQRT live in different ACT table sets, so
                # interleaving them per-st pays a 1.3us table reload each time
                for st in range(NST):
                    for ns in range(DM // N512):
                        ps = pop.tile([NP, N512], F32, tag="po", name="ps")
                        for g2 in range(KC // 2):
                            nc.tensor.matmul(
                                ps,
                                ctxT_sb[:, 2 * g2:2 * g2 + 2,
                                        st * NP:(st + 1) * NP],
                                wgo8_sb[:, g2, :, ns * N512:(ns + 1) * N512],
                                start=(g2 == 0), stop=(g2 == KC // 2 - 1),
                                perf_mode=DR)
                        nc.vector.tensor_add(
                            gate_sb[:, st, ns * N512:(ns + 1) * N512],
                            ps, bgo_bc[:, ns * N512:(ns + 1) * N512])
                    # the 1/(s_c*s_go8) un-scale rides the sigmoid's affine
                    nc.scalar.activation(out=gate_sb[:, st, :],
                                         in_=gate_sb[:, st, :],
                                         func=AF.Sigmoid,
                                         scale=osv_sb[:, 2:3])
                for st in range(NST):
                    for ns in range(DM // N512):
                        ps = pop.tile([NP, N512], F32, tag="po", name="ps")
                        for g2 in range(KC // 2):
                            nc.tensor.matmul(
                                ps,
                                ctxT_sb[:, 2 * g2:2 * g2 + 2,
                                        st * NP:(st + 1) * NP],
                                wo8_sb[:, g2, :, ns * N512:(ns + 1) * N512],
                                start=(g2 == 0), stop=(g2 == KC // 2 - 1),
                                perf_mode=DR)
                        nc.vector.tensor_add(
                            orow_sb[:, st, ns * N512:(ns + 1) * N512],
                            ps, bo_bc[:, ns * N512:(ns + 1) * N512])
                    nc.vector.tensor_scalar_mul(orow_sb[:, st, :],
                                                orow_sb[:, st, :],
                                                osv_sb[:, 1:2])
                    # y_pre = gate*(out - x) + 2x
                    nc.vector.tensor_sub(t1_sb[:, st, :], orow_sb[:, st, :],
                                         xr_sb[:, st, :])
                    nc.vector.tensor_mul(orow_sb[:, st, :], t1_sb[:, st, :],
                                         gate_sb[:, st, :])
                    nc.vector.scalar_tensor_tensor(
                        out=t1_sb[:, st, :], in0=xr_sb[:, st, :], scalar=2.0,
                        in1=orow_sb[:, st, :], op0=ALU.mult, op1=ALU.add)
                    # layernorm over DM
                    yv = t1_sb[:, st, :].rearrange("p (g d) -> p g d", g=2)
                    for g in range(2):
                        nc.vector.bn_stats(out=stats[:, g, :], in_=yv[:, g, :])
                    nc.vector.bn_aggr(out=mv, in_=stats)
                    nc.scalar.activation(out=std, in_=mv[:, 1:2],
                                         func=AF.Sqrt, bias=eps_sb)
                    nc.vector.reciprocal(rstd, std)
                    nc.vector.tensor_scalar(
                        out=orow_sb[:, st, :], in0=t1_sb[:, st, :],
                        scalar1=mv[:, 0:1], scalar2=rstd,
                        op0=ALU.subtract, op1=ALU.mult)
                    nc.vector.tensor_mul(orow_sb[:, st, :], orow_sb[:, st, :],
                                         gam_bc)
                    nc.vector.tensor_add(y_sb[:, st % 2, :], orow_sb[:, st, :],
                                         bet_bc)
                    nc.gpsimd.dma_start(
                        out=y_d.rearrange("(n p) d -> p n d", p=NP)[:, st, :],
                        in_=y_sb[:, st % 2, :])
            wep.release()

    nc.compile()
    return nc


def kernel(x, Wq, bq, Wk, bk, Wv, bv, Wo, bo, Wg, bg, attention_weights,
           ln_gamma, ln_beta):
    x = np.asarray(x, dtype=np.float32)
    f32 = lambda a: np.ascontiguousarray(np.asarray(a, dtype=np.float32))
    bf16 = lambda a: np.ascontiguousarray(
        np.asarray(a, dtype=np.float32)).astype(ml_dtypes.bfloat16)
    Wq, Wk, Wv, Wo, Wg = map(f32, (Wq, Wk, Wv, Wo, Wg))
    bq, bk, bv, bo, bg = map(f32, (bq, bk, bv, bo, bg))
    aw, gam, bet = map(f32, (attention_weights, ln_gamma, ln_beta))

    if "nc" not in _CACHE:
        _CACHE["nc"] = _build()
    nc = _CACHE["nc"]

    # fold softmax(attention_weights) into Wv / bv
    e = np.exp(aw - aw.max())
    head_w = (e / e.sum()).astype(np.float32)
    hw_exp = np.repeat(head_w, HD)              # [DM]
    Wv_s = Wv * hw_exp[:, None]
    bv_s = bv * hw_exp
    # fold Wg @ Wo so the gate projection reads ctx^T directly
    Wgo = (Wg.astype(np.float64) @ Wo.astype(np.float64)).astype(np.float32)
    bgo = (bg.astype(np.float64)
           + Wg.astype(np.float64) @ bo.astype(np.float64)).astype(np.float32)

    # p-major row shuffle: row (c*128+p) -> position (p*KC+c), so each
    # [128, KC, cols] SBUF tile loads with one descriptor per partition
    def pmaj(a):
        n = a.shape[0]
        return np.ascontiguousarray(
            a.reshape(n // NP, NP, *a.shape[1:]).swapaxes(0, 1)
            .reshape(a.shape))

    # DoubleRow row shuffle: contraction row (kc2*256+two*128+p) ->
    # position (p*8 + kc2*2 + two), p-major for 1-descriptor DMAs
    def drmaj(a):
        return np.ascontiguousarray(
            a.reshape(KC // 2, 2, NP, *a.shape[1:])
            .transpose(2, 0, 1, *range(3, a.ndim + 2))
            .reshape(a.shape))

    # power-of-two fp8 scales (float8_e4m3 max finite is 240)
    def p2s(w):
        return float(2.0 ** np.round(np.log2(100.0 / np.abs(w).max())))

    f8 = lambda a: np.ascontiguousarray(a).astype(ml_dtypes.float8_e4m3)
    s_k = p2s(Wk)
    s_v = p2s(Wv_s)
    s_q = p2s(Wq)
    s_o = p2s(Wo)
    s_go = p2s(Wgo)
    S_C = 16.0                      # fp8 scale for ctx (|ctx| ~ O(1))
    wq8 = f8(drmaj(Wq.T * s_q))
    wk8 = f8(drmaj(Wk.T * s_k))
    wv8 = f8(drmaj(Wv_s.T * s_v))
    wo8 = f8(drmaj(Wo.T * s_o))
    wgo8 = f8(drmaj(Wgo.T * s_go))
    gsc = np.array([SCALE / (s_k * s_q)], dtype=np.float32)
    # [ctx-quant scale s_c/s_v, out-proj unscale, gate-proj unscale]
    oscv = np.array([S_C / s_v, 1.0 / (S_C * s_o), 1.0 / (S_C * s_go)],
                    dtype=np.float32)

    in_maps = []
    for c in range(8):
        b, blk = divmod(c, 4)
        r0 = blk * SQ
        xb = x[b]
        perm = np.r_[r0:r0 + SQ, 0:r0, r0 + SQ:S]
        xT = drmaj(xb[perm].T)
        in_maps.append({
            "xT8q": f8(xT[:, 0:SQ]),
            "xT8r": f8(xT[:, SQ:]),
            "xr": bf16(xb[r0:r0 + SQ]),
            "wk8": wk8, "wv8": wv8, "wq8": wq8, "wo8": wo8, "wgo8": wgo8,
            "bq": pmaj(bq * s_q), "bk": pmaj(bk * s_k), "bv": bv_s * s_v,
            "bo": bo * (S_C * s_o), "bgo": bgo * (S_C * s_go),
            "gam": gam, "bet": bet, "gsc": gsc, "oscv": oscv,
        })

    last_exc = None
    for _attempt in range(3):
        try:
            res = run_bass_kernel_spmd(nc, in_maps, core_ids=list(range(8)),
                                       trace=_TRACE[0])
            break
        except Exception as exc:  # flaky NRT_EXEC_UNIT errors: retry
            last_exc = exc
            import time
            time.sleep(2.0)
    else:
        raise last_exc
    _LAST_RESULT[0] = res

    y = np.empty((B, S, DM), dtype=np.float32)
    for c in range(8):
        b, blk = divmod(c, 4)
        r0 = blk * SQ
        y[b, r0:r0 + SQ] = res.results[c]["y"]
    return y


# revision 64
# speedup vs baseline: 1.0346x; 1.0346x over previous
"""Trainium2 Bass kernel for EnhancedMultiHeadAttention (B=2, S=2048, DM=1024, H=16).

Sharding: 8 NeuronCores = 2 batches x 4 query-row blocks of 512 rows. Each
core computes K/V for its whole batch (4x redundant: cheaper than sharing via
collectives, which measured ~45GB/s with heavy serialization on the CC path),
plus attention, output projection, gate and layernorm for its own 512 query
rows. No collectives; the host concatenates the 8 output shards.

All matmuls run in bf16 with fp32 PSUM accumulation (rel err ~8e-4, gate is
2e-2). bf16 halves input DMA bytes, lets fast-weight-load halve LDWEIGHTS,
and lets the row/col tile_position-packed score and attn@v matmul pairs
actually overlap on the PE (fp32r pairs serialized; bf16 pairs run with the
second MM finishing ~30ns after the first).

Schedule: one fused window. Q^T is projected up front; each head pair's K
rows are projected just-in-time; V tiles are projected into SBUF-resident
tiles, with pair 0's V tiles fused into its attention t-loop so the ScalarE
exact-erf GELU stream (~130us serial: 16.8M elems/core) starts as early as
possible. All projection matmuls fill PE slack underneath the GELU stream.
The score scale rides the GELU activation's free affine;
softmax(attention_weights) is folded into Wv/bv on the host; per-core query
columns are permuted to the front of x^T so Q projects from the resident
tile (t-order in attention is permutation-invariant as long as K and V share
it). The gate projection reads ctx^T directly via the host-folded
Wgo = Wg @ Wo (bgo = bg + Wg bo), which removes the transposed
out-projection of the previous version entirely. Epilogue (out proj via
ctx^T, gate, sigmoid blend, bn_stats layernorm) trails per 128-row tile.
"""
import math
import sys

import numpy as np

for _p in ("/opt/trn_rl_repo", "/opt/pypackages"):
    if _p not in sys.path:
        sys.path.append(_p)

import ml_dtypes
import concourse.bass as bass
import concourse.mybir as mybir
import concourse.tile as tile
from concourse import bacc
from concourse.bass_utils import run_bass_kernel_spmd

F32 = mybir.dt.float32
BF16 = mybir.dt.bfloat16
FP8 = mybir.dt.float8e4
DR = mybir.MatmulPerfMode.DoubleRow
AF = mybir.ActivationFunctionType
ALU = mybir.AluOpType

B, S, DM, H = 2, 2048, 1024, 16
HD = DM // H                  # 64
SQ = 512                      # query rows per core
NP = 128                      # partitions
KC = DM // NP                 # 8 contraction chunks
NT = S // NP                  # 16 key/value tiles
NPAIR = H // 2                # 8 head pairs
NST = SQ // NP                # 4 row tiles in epilogue
N512 = 512
NQ = 512                      # v-projection column half width
SCALE = 1.0 / math.sqrt(HD)
EPS = 1e-5

_CACHE = {}
_TRACE = [False]
_LAST_RESULT = [None]


def _bcast(ap_1d, p=NP):
    return bass.AP(tensor=ap_1d.tensor, offset=ap_1d.offset,
                   ap=[[0, p]] + list(ap_1d.ap))


def _build():
    nc = bacc.Bacc("TRN2", target_bir_lowering=False, debug=False)

    # all big inputs are host-laid-out p-major ("(p c) ..." rows) so each
    # DMA needs only one contiguous descriptor per partition
    xT8q_d = nc.dram_tensor("xT8q", [DM, SQ], FP8, kind="ExternalInput").ap()
    xT8r_d = nc.dram_tensor("xT8r", [DM, S - SQ], FP8,
                            kind="ExternalInput").ap()
    xr_d = nc.dram_tensor("xr", [SQ, DM], BF16, kind="ExternalInput").ap()
    wk8_d = nc.dram_tensor("wk8", [DM, DM], FP8, kind="ExternalInput").ap()
    wv8_d = nc.dram_tensor("wv8", [DM, DM], FP8, kind="ExternalInput").ap()
    wq8_d = nc.dram_tensor("wq8", [DM, DM], FP8, kind="ExternalInput").ap()
    gsc_d = nc.dram_tensor("gsc", [1], F32, kind="ExternalInput").ap()
    oscv_d = nc.dram_tensor("oscv", [3], F32, kind="ExternalInput").ap()
    wo8_d = nc.dram_tensor("wo8", [DM, DM], FP8, kind="ExternalInput").ap()
    wgo8_d = nc.dram_tensor("wgo8", [DM, DM], FP8, kind="ExternalInput").ap()
    bq_d = nc.dram_tensor("bq", [DM], F32, kind="ExternalInput").ap()
    bk_d = nc.dram_tensor("bk", [DM], F32, kind="ExternalInput").ap()
    bv_d = nc.dram_tensor("bv", [DM], F32, kind="ExternalInput").ap()
    bo_d = nc.dram_tensor("bo", [DM], F32, kind="ExternalInput").ap()
    bgo_d = nc.dram_tensor("bgo", [DM], F32, kind="ExternalInput").ap()
    gam_d = nc.dram_tensor("gam", [DM], F32, kind="ExternalInput").ap()
    bet_d = nc.dram_tensor("bet", [DM], F32, kind="ExternalInput").ap()
    y_d = nc.dram_tensor("y", [SQ, DM], F32, kind="ExternalOutput").ap()

    # DoubleRow row order (p, kc2, two): contraction row = kc2*256+two*128+p
    xT8q_v = xT8q_d.rearrange("(p k two) s -> p k two s",
                              p=NP, k=KC // 2, two=2)
    xT8r_v = xT8r_d.rearrange("(p k two) s -> p k two s",
                              p=NP, k=KC // 2, two=2)
    wk8_v = wk8_d.rearrange("(p k two) d -> p k two d", p=NP, k=KC // 2, two=2)
    wv8_v = wv8_d.rearrange("(p k two) d -> p k two d", p=NP, k=KC // 2, two=2)
    wq8_v = wq8_d.rearrange("(p k two) d -> p k two d", p=NP, k=KC // 2, two=2)
    wo8_v = wo8_d.rearrange("(p k two) d -> p k two d", p=NP, k=KC // 2, two=2)
    wgo8_v = wgo8_d.rearrange("(p k two) d -> p k two d", p=NP, k=KC // 2, two=2)

    with tile.TileContext(nc) as tc:
        with tc.tile_pool(name="pers", bufs=1) as pers, \
             tc.tile_pool(name="acc", bufs=1) as acc:
            bq_sb = pers.tile([NP, KC], F32)
            bk_sb = pers.tile([NP, KC], F32)
            nc.sync.dma_start(out=bq_sb, in_=bq_d.rearrange("(p c) -> p c", p=NP))
            nc.sync.dma_start(out=bk_sb, in_=bk_d.rearrange("(p c) -> p c", p=NP))
            bv_bc = pers.tile([NP, DM], F32)
            nc.sync.dma_start(out=bv_bc, in_=_bcast(bv_d))
            gsc_sb = pers.tile([NP, 1], F32)
            nc.sync.dma_start(out=gsc_sb, in_=_bcast(gsc_d))
            # [csc, osc, sgsc]: ctx fp8 quantization scale, out-proj unscale,
            # gate-proj unscale (folded into the sigmoid's affine)
            osv_sb = pers.tile([NP, 3], F32)
            nc.sync.dma_start(out=osv_sb, in_=_bcast(oscv_d))
            eps_sb = pers.tile([NP, 1], F32)
            nc.vector.memset(eps_sb, EPS)

            ctxT_sb = acc.tile([NP, NPAIR, SQ], FP8)

            wep = tc.alloc_tile_pool(name="wep", bufs=1)
            wo8_sb = wep.tile([NP, KC // 2, 2, DM], FP8)
            wgo8_sb = wep.tile([NP, KC // 2, 2, DM], FP8)
            xr_sb = wep.tile([NP, NST, DM], BF16)
            bo_bc = wep.tile([NP, DM], F32)
            bgo_bc = wep.tile([NP, DM], F32)
            gam_bc = wep.tile([NP, DM], F32)
            bet_bc = wep.tile([NP, DM], F32)

            with tc.tile_pool(name="xres", bufs=1) as xres, \
                 tc.tile_pool(name="qres", bufs=1) as qres, \
                 tc.tile_pool(name="wts", bufs=1) as wts, \
                 tc.tile_pool(name="kpp", bufs=2) as kpp, \
                 tc.tile_pool(name="vqp", bufs=1) as vqp, \
                 tc.tile_pool(name="attp", bufs=3) as attp, \
                 tc.tile_pool(name="pp", bufs=2, space="PSUM") as pp, \
                 tc.tile_pool(name="scop", bufs=2, space="PSUM") as scop, \
                 tc.tile_pool(name="cxp", bufs=2, space="PSUM") as cxp:
                x8q_sb = xres.tile([NP, KC // 2, 2, SQ], FP8)
                x8r_sb = xres.tile([NP, KC // 2, 2, S - SQ], FP8)
                qT_sb = qres.tile([NP, KC, SQ], BF16)
                wq8_sb = wts.tile([NP, KC // 2, 2, DM], FP8)
                wk8_sb = wts.tile([NP, KC // 2, 2, DM], FP8)
                wv8_sb = wts.tile([NP, KC // 2, 2, DM], FP8)
                vq = vqp.tile([NP, NT, DM], BF16)

                def x8_t(k2, tt):
                    if tt < NST:
                        return x8q_sb[:, k2, :, tt * NP:(tt + 1) * NP]
                    o = (tt - NST) * NP
                    return x8r_sb[:, k2, :, o:o + NP]

                def x8_s(k2, ts):
                    if ts == 0:
                        return x8q_sb[:, k2, :, :]
                    o = (ts - 1) * N512
                    return x8r_sb[:, k2, :, o:o + N512]

                def v_tt(tt, ns):
                    # v'[t, d] = sum_k x[t, k] (s_v Wv[d, k]), fp8 DoubleRow
                    ps_t = pp.tile([NP, NQ], F32, tag="pj", name="ps_t")
                    for k2 in range(KC // 2):
                        nc.tensor.matmul(
                            ps_t,
                            x8_t(k2, tt),
                            wv8_sb[:, k2, :, ns * NQ:(ns + 1) * NQ],
                            start=(k2 == 0), stop=(k2 == KC // 2 - 1),
                            perf_mode=DR)
                    nc.vector.tensor_add(
                        vq[:, tt, ns * NQ:(ns + 1) * NQ], ps_t,
                        bv_bc[:, ns * NQ:(ns + 1) * NQ])

                kpairs = [None] * NPAIR

                def prep_ts(p, ts):
                    # one 512-col slab of kpair[d, t] = s_k(Wk x^T + bk)
                    if ts == 0:
                        kpairs[p] = kpp.tile([NP, S], BF16, tag="kp",
                                             name="kpair")
                    kpair = kpairs[p]
                    ps_t = pp.tile([NP, N512], F32, tag="pj", name="ps_t")
                    for k2 in range(KC // 2):
                        nc.tensor.matmul(
                            ps_t,
                            wk8_sb[:, k2, :, p * NP:(p + 1) * NP],
                            x8_s(k2, ts),
                            start=(k2 == 0), stop=(k2 == KC // 2 - 1),
                            perf_mode=DR)
                    nc.vector.tensor_scalar_add(
                        kpair[:, ts * N512:(ts + 1) * N512], ps_t,
                        bk_sb[:, p:p + 1])

                def qproj(dt):
                    ps_q = pp.tile([NP, SQ], F32, tag="pj", name="ps_q")
                    for k2 in range(KC // 2):
                        nc.tensor.matmul(ps_q,
                                         wq8_sb[:, k2, :, dt * NP:(dt + 1) * NP],
                                         x8q_sb[:, k2, :, :],
                                         start=(k2 == 0),
                                         stop=(k2 == KC // 2 - 1),
                                         perf_mode=DR)
                    nc.vector.tensor_scalar_add(qT_sb[:, dt, :], ps_q,
                                                bq_sb[:, dt:dt + 1])

                def attn(p, pre_t=None, fill=()):
                    # fill: projection-chain closures for FUTURE pairs,
                    # dispensed evenly across the t-loop so the in-order PE
                    # queue never starves the GELU stream at pair boundaries
                    fill = list(fill)
                    n0 = len(fill)
                    kpair = kpairs[p]
                    c0 = p * NP
                    ctx_ps = cxp.tile([NP, SQ], F32, tag="cx", name="ctx_ps")
                    for t in range(NT):
                        while len(fill) > n0 * (NT - 1 - t) // NT:
                            fill.pop(0)()
                        if pre_t is not None:
                            pre_t(t)
                        sco = scop.tile([NP, 2 * SQ], F32, tag="sc", name="sco")
                        nc.tensor.matmul(sco[:, 0:SQ],
                                         kpair[0:64, t * NP:(t + 1) * NP],
                                         qT_sb[0:64, p, :],
                                         start=True, stop=True,
                                         tile_position=(0, 0))
                        nc.tensor.matmul(sco[:, SQ:2 * SQ],
                                         kpair[64:NP, t * NP:(t + 1) * NP],
                                         qT_sb[64:NP, p, :],
                                         start=True, stop=True,
                                         tile_position=(64, 0))
                        att_t = attp.tile([NP, 2 * SQ], BF16, tag="at", name="att_t")
                        nc.scalar.activation(out=att_t, in_=sco, func=AF.Gelu,
                                             scale=gsc_sb[:, 0:1])
                        nc.tensor.matmul(ctx_ps[0:64, :], vq[:, t, c0:c0 + 64],
                                         att_t[:, 0:SQ],
                                         start=(t == 0), stop=(t == NT - 1),
                                         tile_position=(0, 0))
                        nc.tensor.matmul(ctx_ps[64:NP, :],
                                         vq[:, t, c0 + 64:c0 + NP],
                                         att_t[:, SQ:2 * SQ],
                                         start=(t == 0), stop=(t == NT - 1),
                                         tile_position=(0, 64))
                    # rescale s_v*ctx -> s_c*ctx so it fits fp8 for the
                    # DoubleRow out/gate projections
                    nc.vector.tensor_scalar_mul(ctxT_sb[:, p, :], ctx_ps,
                                                osv_sb[:, 0:1])

                # opening DMA order by need-time: Q-proj deps (query cols +
                # wq) first, then wk (pair 0's K), xT rest, wv. The epilogue
                # prefetch (wo/wgo/xr, needed only ~250us in) is issued from
                # the scalar queue mid-attention so its descriptors don't
                # contend with the critical loads.
                # single queue (sync/HWDGE) in exact need order, so the
                # per-queue descriptor FIFOs preserve the priority
                nc.sync.dma_start(out=wq8_sb, in_=wq8_v)
                nc.sync.dma_start(out=x8q_sb, in_=xT8q_v)
                nc.sync.dma_start(out=wk8_sb, in_=wk8_v)
                nc.sync.dma_start(out=x8r_sb, in_=xT8r_v)
                nc.sync.dma_start(out=wv8_sb, in_=wv8_v)

                # only pair 0's projections run up front; everything else is
                # dispensed as fillers inside earlier pairs' t-loops
                def mk(f, *a):
                    return lambda: f(*a)

                qproj(0)
                for _ts in range(4):
                    prep_ts(0, _ts)
                attn(0, pre_t=lambda t: v_tt(t, 0),
                     fill=[mk(qproj, 1)] + [mk(prep_ts, 1, ts)
                                            for ts in range(4)])
                for p in range(1, NPAIR):
                    fl = []
                    if 1 <= p <= 4:
                        fl += [mk(v_tt, tt, 1)
                               for tt in range(4 * (p - 1), 4 * p)]
                    if p + 1 < NPAIR:
                        fl += [mk(qproj, p + 1)] + [mk(prep_ts, p + 1, ts)
                                                    for ts in range(4)]
                    if p == 4:
                        # epilogue prefetch: issued on the scalar queue, deep
                        # in the GELU stream by now — a natural throttle that
                        # keeps these loads off the DMA engines early on
                        nc.scalar.dma_start(out=wo8_sb, in_=wo8_v)
                        nc.scalar.dma_start(out=wgo8_sb, in_=wgo8_v)
                        nc.scalar.dma_start(
                            out=xr_sb,
                            in_=xr_d.rearrange("(n p) d -> p n d", p=NP))
                        nc.scalar.dma_start(out=bo_bc, in_=_bcast(bo_d))
                        nc.scalar.dma_start(out=bgo_bc, in_=_bcast(bgo_d))
                        nc.scalar.dma_start(out=gam_bc, in_=_bcast(gam_d))
                        nc.scalar.dma_start(out=bet_bc, in_=_bcast(bet_d))
                    attn(p, fill=fl)

            # ------------- out proj, gate + epilogue -----------------------
            with tc.tile_pool(name="big", bufs=1) as big, \
                 tc.tile_pool(name="pop", bufs=4, space="PSUM") as pop:
                orow_sb = big.tile([NP, NST, DM], BF16)
                gate_sb = big.tile([NP, NST, DM], BF16)
                t1_sb = big.tile([NP, NST, DM], BF16)
                y_sb = big.tile([NP, 2, DM], F32)
                stats = pers.tile([NP, 2, 6], F32)
                mv = pers.tile([NP, 2], F32)
                std = pers.tile([NP, 1], F32)
                rstd = pers.tile([NP, 1], F32)
                # all gate matmuls + sigmoids first, then the out-proj/LN
                # pass: SIGMOID and SQRT live in different ACT table sets, so
                # interleaving them per-st pays a 1.3us table reload each time
                for st in range(NST):
                    for ns in range(DM // N512):
                        ps = pop.tile([NP, N512], F32, tag="po", name="ps")
                        for g2 in range(KC // 2):
                            nc.tensor.matmul(
                                ps,
                                ctxT_sb[:, 2 * g2:2 * g2 + 2,
                                        st * NP:(st + 1) * NP],
                                wgo8_sb[:, g2, :, ns * N512:(ns + 1) * N512],
                                start=(g2 == 0), stop=(g2 == KC // 2 - 1),
                                perf_mode=DR)
                        nc.vector.tensor_add(
                            gate_sb[:, st, ns * N512:(ns + 1) * N512],
                            ps, bgo_bc[:, ns * N512:(ns + 1) * N512])
                    # the 1/(s_c*s_go8) un-scale rides the sigmoid's affine
                    nc.scalar.activation(out=gate_sb[:, st, :],
                                         in_=gate_sb[:, st, :],
                                         func=AF.Sigmoid,
                                         scale=osv_sb[:, 2:3])
                for st in range(NST):
                    for ns in range(DM // N512):
                        ps = pop.tile([NP, N512], F32, tag="po", name="ps")
                        for g2 in range(KC // 2):
                            nc.tensor.matmul(
                                ps,
                                ctxT_sb[:, 2 * g2:2 * g2 + 2,
                                        st * NP:(st + 1) * NP],
                                wo8_sb[:, g2, :, ns * N512:(ns + 1) * N512],
                                start=(g2 == 0), stop=(g2 == KC // 2 - 1),
                                perf_mode=DR)
                        nc.vector.tensor_add(
                            orow_sb[:, st, ns * N512:(ns + 1) * N512],
                            ps, bo_bc[:, ns * N512:(ns + 1) * N512])
                    nc.vector.tensor_scalar_mul(orow_sb[:, st, :],
                                                orow_sb[:, st, :],
                                                osv_sb[:, 1:2])
                    # y_pre = gate*(out - x) + 2x
                    nc.vector.tensor_sub(t1_sb[:, st, :], orow_sb[:, st, :],
                                         xr_sb[:, st, :])
                    nc.vector.tensor_mul(orow_sb[:, st, :], t1_sb[:, st, :],
                                         gate_sb[:, st, :])
                    nc.vector.scalar_tensor_tensor(
                        out=t1_sb[:, st, :], in0=xr_sb[:, st, :], scalar=2.0,
                        in1=orow_sb[:, st, :], op0=ALU.mult, op1=ALU.add)
                    # layernorm over DM
                    yv = t1_sb[:, st, :].rearrange("p (g d) -> p g d", g=2)
                    for g in range(2):
                        nc.vector.bn_stats(out=stats[:, g, :], in_=yv[:, g, :])
                    nc.vector.bn_aggr(out=mv, in_=stats)
                    nc.scalar.activation(out=std, in_=mv[:, 1:2],
                                         func=AF.Sqrt, bias=eps_sb)
                    nc.vector.reciprocal(rstd, std)
                    nc.vector.tensor_scalar(
                        out=orow_sb[:, st, :], in0=t1_sb[:, st, :],
                        scalar1=mv[:, 0:1], scalar2=rstd,
                        op0=ALU.subtract, op1=ALU.mult)
                    nc.vector.tensor_mul(orow_sb[:, st, :], orow_sb[:, st, :],
                                         gam_bc)
                    nc.vector.tensor_add(y_sb[:, st % 2, :], orow_sb[:, st, :],
                                         bet_bc)
                    nc.gpsimd.dma_start(
                        out=y_d.rearrange("(n p) d -> p n d", p=NP)[:, st, :],
                        in_=y_sb[:, st % 2, :])
            wep.release()

    nc.compile()
    return nc


def kernel(x, Wq, bq, Wk, bk, Wv, bv, Wo, bo, Wg, bg, attention_weights,
           ln_gamma, ln_beta):
    x = np.asarray(x, dtype=np.float32)
    f32 = lambda a: np.ascontiguousarray(np.asarray(a, dtype=np.float32))
    bf16 = lambda a: np.ascontiguousarray(
        np.asarray(a, dtype=np.float32)).astype(ml_dtypes.bfloat16)
    Wq, Wk, Wv, Wo, Wg = map(f32, (Wq, Wk, Wv, Wo, Wg))
    bq, bk, bv, bo, bg = map(f32, (bq, bk, bv, bo, bg))
    aw, gam, bet = map(f32, (attention_weights, ln_gamma, ln_beta))

    if "nc" not in _CACHE:
        _CACHE["nc"] = _build()
    nc = _CACHE["nc"]

    # fold softmax(attention_weights) into Wv / bv
    e = np.exp(aw - aw.max())
    head_w = (e / e.sum()).astype(np.float32)
    hw_exp = np.repeat(head_w, HD)              # [DM]
    Wv_s = Wv * hw_exp[:, None]
    bv_s = bv * hw_exp
    # fold Wg @ Wo so the gate projection reads ctx^T directly
    Wgo = (Wg.astype(np.float64) @ Wo.astype(np.float64)).astype(np.float32)
    bgo = (bg.astype(np.float64)
           + Wg.astype(np.float64) @ bo.astype(np.float64)).astype(np.float32)

    # p-major row shuffle: row (c*128+p) -> position (p*KC+c), so each
    # [128, KC, cols] SBUF tile loads with one descriptor per partition
    def pmaj(a):
        n = a.shape[0]
        return np.ascontiguousarray(
            a.reshape(n // NP, NP, *a.shape[1:]).swapaxes(0, 1)
            .reshape(a.shape))

    # DoubleRow row shuffle: contraction row (kc2*256+two*128+p) ->
    # position (p*8 + kc2*2 + two), p-major for 1-descriptor DMAs
    def drmaj(a):
        return np.ascontiguousarray(
            a.reshape(KC // 2, 2, NP, *a.shape[1:])
            .transpose(2, 0, 1, *range(3, a.ndim + 2))
            .reshape(a.shape))

    # power-of-two fp8 scales (float8_e4m3 max finite is 240)
    def p2s(w):
        return float(2.0 ** np.round(np.log2(100.0 / np.abs(w).max())))

    f8 = lambda a: np.ascontiguousarray(a).astype(ml_dtypes.float8_e4m3)
    s_k = p2s(Wk)
    s_v = p2s(Wv_s)
    s_q = p2s(Wq)
    s_o = p2s(Wo)
    s_go = p2s(Wgo)
    S_C = 16.0                      # fp8 scale for ctx (|ctx| ~ O(1))
    wq8 = f8(drmaj(Wq.T * s_q))
    wk8 = f8(drmaj(Wk.T * s_k))
    wv8 = f8(drmaj(Wv_s.T * s_v))
    wo8 = f8(drmaj(Wo.T * s_o))
    wgo8 = f8(drmaj(Wgo.T * s_go))
    gsc = np.array([SCALE / (s_k * s_q)], dtype=np.float32)
    # [ctx-quant scale s_c/s_v, out-proj unscale, gate-proj unscale]
    oscv = np.array([S_C / s_v, 1.0 / (S_C * s_o), 1.0 / (S_C * s_go)],
                    dtype=np.float32)

    in_maps = []
    for c in range(8):
        b, blk = divmod(c, 4)
        r0 = blk * SQ
        xb = x[b]
        perm = np.r_[r0:r0 + SQ, 0:r0, r0 + SQ:S]
        xT = drmaj(xb[perm].T)
        in_maps.append({
            "xT8q": f8(xT[:, 0:SQ]),
            "xT8r": f8(xT[:, SQ:]),
            "xr": bf16(xb[r0:r0 + SQ]),
            "wk8": wk8, "wv8": wv8, "wq8": wq8, "wo8": wo8, "wgo8": wgo8,
            "bq": pmaj(bq * s_q), "bk": pmaj(bk * s_k), "bv": bv_s * s_v,
            "bo": bo * (S_C * s_o), "bgo": bgo * (S_C * s_go),
            "gam": gam, "bet": bet, "gsc": gsc, "oscv": oscv,
        })

    last_exc = None
    for _attempt in range(3):
        try:
            res = run_bass_kernel_spmd(nc, in_maps, core_ids=list(range(8)),
                                       trace=_TRACE[0])
            break
        except Exception as exc:  # flaky NRT_EXEC_UNIT errors: retry
            last_exc = exc
            import time
            time.sleep(2.0)
    else:
        raise last_exc
    _LAST_RESULT[0] = res

    y = np.empty((B, S, DM), dtype=np.float32)
    for c in range(8):
        b, blk = divmod(c, 4)
        r0 = blk * SQ
        y[b, r0:r0 + SQ] = res.results[c]["y"]
    return y
